# revision 1
# baseline (speedup 1.0000x reference)
"""Trainium2 Bass kernel for AggregationEncoder (gather + scatter-mean GNN encoder).

Computes, per batch b:
    out[b, m, :] = mean over edges e with dst[b,e]==m of grid[b, src[b,e], :]

Sharding: 8 cores = 4 batches x 2 mesh-node halves (disjoint outputs, no
cross-core combine).

Device algorithm per core:
  - Host buckets the core's edges by destination mesh tile (128 mesh rows per
    tile, NT=41 tiles); within each tile's bucket, edges with src < 32768 come
    first (int16 gather indices), each section padded to a multiple of 128.
  - Per mesh tile: dma_gather (GPSIMD extended instruction; indices int16
    wrapped [16, n/16] and replicated to 128 partitions; <=1024 idxs/op;
    rotated across 4 SWDGE queues for parallel descriptor generation)
    pulls the grid rows -> one-hot(dst_local) via is_equal on DVE ->
    accumulating matmuls in PSUM (PE performs the scatter-add) ->
    multiply by host-computed 1/count -> DMA out.
"""
import sys

sys.path.insert(0, '/opt/trn_rl_repo')
import numpy as np

B, G, F, M, E = 4, 65160, 128, 10242, 262144
P = 128
HALF = 5120           # even cores: mesh rows [0, 5120); odd: [5120, 10242)
NT = 41               # mesh tiles per core (SPMD-uniform)
N_CORES = 8
SPLIT = 32768         # int16 gather-index limit: grid rows [0,SPLIT) / [SPLIT,G)
CHUNK_BLOCKS = 8      # max 1024 idxs per dma_gather op

_nc_cache = {}


def _chunks(j0, j1, step=CHUNK_BLOCKS):
    """Balanced partition of [j0, j1) into near-equal chunks of <= step blocks."""
    n = j1 - j0
    if n <= 0:
        return []
    k = -(-n // step)
    base, rem = divmod(n, k)
    out = []
    s = j0
    for i in range(k):
        e = s + base + (1 if i < rem else 0)
        out.append((s, e))
        s = e
    return out


def _build_nc(K_LO, K_HI, maxc_lo, maxc_hi):
    from concourse import bacc
    import concourse.mybir as mybir
    import concourse.tile as tile

    K = K_LO + K_HI
    DT = mybir.dt.float32
    F16 = mybir.dt.float16
    i16 = mybir.dt.int16
    nc = bacc.Bacc(None, target_bir_lowering=False, num_swdge_queues=4)
    grid_d = nc.dram_tensor("grid", [G, F], DT, kind="ExternalInput")
    idx_d = nc.dram_tensor("idx16", [P, NT * K * 8], i16, kind="ExternalInput")
    dl_d = nc.dram_tensor("dl_all", [P, NT * K], DT, kind="ExternalInput")
    inv_d = nc.dram_tensor("inv_all", [P, NT], DT, kind="ExternalInput")
    iota_d = nc.dram_tensor("iota", [P, P], DT, kind="ExternalInput")
    out_d = nc.dram_tensor("out", [NT, P, F], DT, kind="ExternalOutput")

    qn = [0]

    def next_q(n):
        q = qn[0] % 4
        qn[0] += 1
        return q

    with tile.TileContext(nc) as tc:
        with (
            tc.tile_pool(name="const", bufs=1) as cpool,
            tc.tile_pool(name="gath", bufs=7) as gpool,
            tc.tile_pool(name="oneh", bufs=3) as opool,
            tc.tile_pool(name="ostg", bufs=3) as spool,
            tc.tile_pool(name="psum", bufs=5, space="PSUM") as ppool,
        ):
            NA = 2  # head groups get their own small idx tile (fast first load)
            idx_a = cpool.tile([P, NA * K * 8], i16)
            idx_b = cpool.tile([P, (NT - NA) * K * 8], i16)
            dl_t = cpool.tile([P, NT * K], DT)
            inv_t = cpool.tile([P, NT], DT)
            iota_t = cpool.tile([P, P], DT)
            nc.sync.dma_start(idx_a[:], idx_d[:, :NA * K * 8])
            nc.sync.dma_start(idx_b[:], idx_d[:, NA * K * 8:])
            nc.sync.dma_start(dl_t[:], dl_d[:])
            nc.sync.dma_start(inv_t[:], inv_d[:])
            nc.sync.dma_start(iota_t[:], iota_d[:])

            # warm the gather-pool slots so tiles trimmed below the block
            # capacity never expose uninitialized SBUF (dl=-1 zeroes their
            # one-hot rows, but NaN * 0 would still poison PSUM)
            for _w in range(7):
                gw = gpool.tile([P, K, F], DT, tag="g")
                nc.vector.memset(gw[:], 0.0)

            for p in range(NT):
                g = gpool.tile([P, K, F], DT, tag="g")
                idx_t = idx_a if p < NA else idx_b
                base = (p if p < NA else p - NA) * K * 8
                for (j0, j1) in _chunks(0, K_LO):
                    cap = maxc_lo[p]  # static max valid idxs in this section
                    n = min((j1 - j0) * P, max(cap - j0 * P, 0))
                    n = -(-n // 16) * 16
                    if n == 0:
                        continue
                    nb = -(-n // P)
                    nc.gpsimd.dma_gather(
                        out_ap=g[:, j0:j0 + nb, :], in_ap=grid_d[0:SPLIT],
                        idxs_ap=idx_t[:, base + j0 * 8:base + j0 * 8 + n // 16],
                        num_idxs=n, num_idxs_reg=n, elem_size=F,
                        queue_num=next_q(n))
                for (j0, j1) in _chunks(K_LO, K):
                    cap = maxc_hi[p]
                    jr = j0 - K_LO
                    n = min((j1 - j0) * P, max(cap - jr * P, 0))
                    n = -(-n // 16) * 16
                    if n == 0:
                        continue
                    nb = -(-n // P)
                    nc.gpsimd.dma_gather(
                        out_ap=g[:, j0:j0 + nb, :], in_ap=grid_d[SPLIT:G],
                        idxs_ap=idx_t[:, base + j0 * 8:base + j0 * 8 + n // 16],
                        num_idxs=n, num_idxs_reg=n, elem_size=F,
                        queue_num=next_q(n))
                oh = opool.tile([P, K, P], DT, tag="oh")
                nc.vector.tensor_tensor(
                    out=oh[:],
                    in0=dl_t[:, p * K:(p + 1) * K].to_broadcast([P, K, P]),
                    in1=iota_t[:, None, :].to_broadcast([P, K, P]),
                    op=mybir.AluOpType.is_equal,
                )
                ps = ppool.tile([P, F], DT, tag="ps")
                for j in range(K):
                    nc.tensor.matmul(
                        ps[:], lhsT=oh[:, j, :], rhs=g[:, j, :],
                        start=(j == 0), stop=(j == K - 1),
                    )
                ost = spool.tile([P, F], DT, tag="ost")
                nc.vector.tensor_tensor(
                    out=ost[:], in0=ps[:],
                    in1=inv_t[:, p:p + 1].to_broadcast([P, F]),
                    op=mybir.AluOpType.mult,
                )
                nc.sync.dma_start(out_d[p], ost[:])

    nc.compile()
    return nc


def _core_counts(src_b, dst_b, lo, hi):
    """Per-group lo/hi-src edge counts for K sizing."""
    sel = (dst_b >= lo) & (dst_b < hi)
    gt = (dst_b[sel] - lo) >> 7
    is_hi = (src_b[sel] >= SPLIT).astype(np.int64)
    cnt = np.bincount(gt * 2 + is_hi, minlength=NT * 2)
    return cnt[0::2], cnt[1::2]


def _prep_core(src_b, dst_b, lo, hi, K_LO, K_HI):
    K = K_LO + K_HI
    sel = (dst_b >= lo) & (dst_b < hi)
    rel = (dst_b[sel] - lo).astype(np.int64)
    ss = src_b[sel].astype(np.int64)
    gt = rel >> 7
    is_hi = (ss >= SPLIT).astype(np.int64)
    sect = gt * 2 + is_hi
    cnt = np.bincount(sect, minlength=NT * 2)
    order = np.argsort(sect, kind='stable')
    sects = sect[order]
    rels = rel[order]
    sss = ss[order]
    starts = np.zeros(NT * 2, np.int64)
    starts[1:] = np.cumsum(cnt)[:-1]
    pos = np.arange(len(sects)) - starts[sects]
    grp = sects >> 1
    hi_flag = sects & 1
    slot = grp * (K * P) + hi_flag * (K_LO * P) + pos
    idx_flat = np.zeros(NT * K * P, np.int64)
    dl_flat = np.full(NT * K * P, -1.0, np.float32)
    idx_flat[slot] = sss - hi_flag * SPLIT
    dl_flat[slot] = (rels & 127).astype(np.float32)
    dl_all = np.ascontiguousarray(dl_flat.reshape(NT * K, P).T)
    idx16 = idx_flat.astype(np.int16).reshape(NT * K * 8, 16).T  # [16, NT*K*8]
    idx16_all = np.ascontiguousarray(np.tile(idx16, (8, 1)))
    cntrow = np.bincount(rel, minlength=NT * P).astype(np.float32)
    inv_all = np.ascontiguousarray(
        (1.0 / np.maximum(cntrow, 1.0)).reshape(NT, P).T.astype(np.float32))
    return idx16_all, dl_all, inv_all


def _prepare(grid_node_features, edge_index):
    grid_node_features = np.asarray(grid_node_features, dtype=np.float32)
    edge_index = np.asarray(edge_index)
    src = edge_index[..., 0].astype(np.int64)
    dst = edge_index[..., 1].astype(np.int64)

    K_LO = K_HI = 1
    all_lo = np.zeros((N_CORES, NT), np.int64)
    all_hi = np.zeros((N_CORES, NT), np.int64)
    for c in range(N_CORES):
        b, h = c // 2, c % 2
        lo, hi = (0, HALF) if h == 0 else (HALF, M)
        c_lo, c_hi = _core_counts(src[b], dst[b], lo, hi)
        all_lo[c], all_hi[c] = c_lo, c_hi
        K_LO = max(K_LO, int(-(-c_lo.max() // P)))
        K_HI = max(K_HI, int(-(-c_hi.max() // P)))
    maxc_lo = tuple(int(x) for x in all_lo.max(axis=0))
    maxc_hi = tuple(int(x) for x in all_hi.max(axis=0))

    iota_np = np.tile(np.arange(P, dtype=np.float32), (P, 1))
    in_maps = []
    for c in range(N_CORES):
        b, h = c // 2, c % 2
        lo, hi = (0, HALF) if h == 0 else (HALF, M)
        idx16_all, dl_all, inv_all = _prep_core(src[b], dst[b], lo, hi, K_LO, K_HI)
        in_maps.append({
            "grid": np.ascontiguousarray(grid_node_features[b]),
            "idx16": idx16_all,
            "dl_all": dl_all,
            "inv_all": inv_all,
            "iota": iota_np,
        })
    return K_LO, K_HI, maxc_lo, maxc_hi, in_maps


def _assemble(results):
    out = np.zeros((B, M, F), dtype=np.float32)
    for c in range(N_CORES):
        b, h = c // 2, c % 2
        lo, hi = (0, HALF) if h == 0 else (HALF, M)
        block = np.asarray(results[c]["out"]).reshape(NT * P, F)
        out[b, lo:hi] = block[:hi - lo]
    return out


def run(grid_node_features, edge_index, trace=False, tmpdir=None):
    from concourse.bass_utils import run_bass_kernel_spmd

    K_LO, K_HI, maxc_lo, maxc_hi, in_maps = _prepare(grid_node_features, edge_index)
    key = (K_LO, K_HI, maxc_lo, maxc_hi)
    if key not in _nc_cache:
        _nc_cache[key] = _build_nc(K_LO, K_HI, maxc_lo, maxc_hi)
    nc = _nc_cache[key]
    res = run_bass_kernel_spmd(
        nc, in_maps, list(range(N_CORES)), trace=trace, tmpdir=tmpdir)
    return _assemble(res.results), res


def kernel(grid_node_features, edge_index):
    out, _ = run(grid_node_features, edge_index)
    return out

